# revision 1
# baseline (speedup 1.0000x reference)
"""ChamferIndex kernel for Trainium2 (8 NeuronCores).

For each batch b (8 total) and each direction:
  idx[b, i] = argmin_j ||a[b,i] - c[b,j]||^2   (first-min tie-break)

Sharding: batch-parallel -- core b handles batch b, both directions.

Per-core algorithm (per direction, 16384 queries x 16384 refs):
  score s[i,j] = 2*x_i . y_j - ||y_j||^2  (argmax s == argmin dist; the
  ||x_i||^2 term is constant per row and dropped).
  - TensorE: K=4 fp32 matmuls [x0,x1,x2,1].T @ [2y0,2y1,2y2,-yn], row-packed
    4x via tile_position into 4 PSUM banks (2048-wide chunks).
  - ScalarE: copies even columns PSUM->SBUF (s_even).
  - VectorE: tensor_tensor_reduce(max, max) over (odd columns PSUM, s_even)
    -> pairwise-max array l1 in SBUF + chained row max (chunkmax).
  - VectorE max_index over l1 finds the first pair-position p* achieving the
    row max; GPSIMD indirect_copy + diagonal-mask dot fetches s_even[p*] per
    row to resolve the even/odd bit b. j* = 2*p* + b, exact first-occurrence.
"""

import numpy as np

NQ = 16384   # queries per task
NR = 16384   # references per task
CHUNK = 2048
NCH = NR // CHUNK          # 8 psum chunks per block
HALF = CHUNK // 2          # 1024
NBLK = NQ // 128           # 128 i-blocks per task
NEG = -3.4e38

_cache = {}


def _build_nc(unroll=0):
    import concourse.bass as bass
    import concourse.mybir as mybir
    import concourse.tile as tile
    from concourse import bacc

    f32 = mybir.dt.float32
    nc = bacc.Bacc("TRN2", target_bir_lowering=False, debug=False)

    ins = {}
    outs = {}
    for t in (1, 2):
        ins[t] = (
            nc.dram_tensor(f"xa{t}", [4, NQ], f32, kind="ExternalInput"),
            nc.dram_tensor(f"ya{t}", [4, NR], f32, kind="ExternalInput"),
        )
        outs[t] = nc.dram_tensor(f"idx{t}", [NQ], mybir.dt.int32, kind="ExternalOutput")
    diagm_d = nc.dram_tensor("diagm", [128, 16], f32, kind="ExternalInput")

    with tile.TileContext(nc) as tc:
        with (
            tc.tile_pool(name="const", bufs=1) as cpool,
            tc.tile_pool(name="ya", bufs=1) as yapool,
            tc.tile_pool(name="xa", bufs=2) as xapool,
            tc.tile_pool(name="se", bufs=2) as sepool,
            tc.tile_pool(name="l1p", bufs=1) as l1pool,
            tc.tile_pool(name="small", bufs=2) as sm,
            tc.tile_pool(name="ps", bufs=2, space="PSUM") as ps,
        ):
            # diag mask M[p, i] = (i == p % 16), host-provided constant
            diagm = cpool.tile([128, 16], f32)
            nc.sync.dma_start(diagm[:], diagm_d[:, :])

            for t in (1, 2):
                xa_d, ya_d = ins[t]
                idx_flat = outs[t]

                # resident ya, replicated at partition offsets 0/32/64/96
                ya_rep = yapool.tile([128, NR], f32, tag="ya")
                for r in range(4):
                    nc.sync.dma_start(ya_rep[32 * r:32 * r + 4, :], ya_d[:, :])

                def body(iv):
                    xa_rep = xapool.tile([128, 128], f32, tag="xa")
                    for r in range(4):
                        nc.sync.dma_start(xa_rep[32 * r:32 * r + 4, :],
                                          xa_d[:, bass.ts(iv, 128)])
                    s_even = sepool.tile([128, NR // 2], f32, tag="se")
                    l1 = l1pool.tile([128, NR // 2], f32, tag="l1")
                    for c in range(NCH):
                        pt = ps.tile([128, CHUNK], f32, tag="pt")
                        for r in range(4):
                            nc.tensor.matmul(
                                pt[:, 512 * r:512 * (r + 1)],
                                xa_rep[32 * r:32 * r + 4, :],
                                ya_rep[32 * r:32 * r + 4,
                                       CHUNK * c + 512 * r:CHUNK * c + 512 * (r + 1)],
                                start=True, stop=True,
                                tile_position=(32 * r, 0),
                            )
                        nc.scalar.copy(s_even[:, HALF * c:HALF * (c + 1)],
                                       pt[:, 0:CHUNK:2])
                        nc.vector.tensor_tensor(
                            out=l1[:, HALF * c:HALF * (c + 1)],
                            in0=pt[:, 1:CHUNK:2],
                            in1=s_even[:, HALF * c:HALF * (c + 1)],
                            op=mybir.AluOpType.max,
                        )
                    top8 = sm.tile([128, 8], f32, tag="g8")
                    nc.vector.max(top8[:], l1[:])
                    gmax = top8[:, 0:1]
                    p8 = sm.tile([128, 8], mybir.dt.uint32, tag="p8")
                    nc.vector.max_index(p8[:], top8[:], l1[:])
                    p16 = sm.tile([128, 1], mybir.dt.uint16, tag="p16")
                    nc.vector.tensor_copy(p16[:], p8[:, 0:1])
                    out16 = sm.tile([128, 16], f32, tag="o16")
                    nc.gpsimd.indirect_copy(out16[:], s_even[:], p16[:], True)
                    v_even = sm.tile([128, 1], f32, tag="ve")
                    junk16 = sm.tile([128, 16], f32, tag="j16")
                    nc.vector.tensor_tensor(out=junk16[:], in0=out16[:],
                                            in1=diagm[:], op=mybir.AluOpType.mult)
                    nc.vector.reduce_sum(v_even[:], junk16[:],
                                         axis=mybir.AxisListType.X)
                    eq = sm.tile([128, 1], f32, tag="eq")
                    nc.vector.tensor_tensor(out=eq[:], in0=v_even[:], in1=gmax,
                                            op=mybir.AluOpType.is_equal)
                    pf = sm.tile([128, 1], f32, tag="pf")
                    nc.vector.tensor_copy(pf[:], p8[:, 0:1])
                    jf = sm.tile([128, 1], f32, tag="jf")
                    # j = 2p + 1 - eq
                    nc.vector.tensor_scalar(out=jf[:], in0=pf[:], scalar1=2.0,
                                            scalar2=1.0, op0=mybir.AluOpType.mult,
                                            op1=mybir.AluOpType.add)
                    nc.vector.tensor_tensor(out=jf[:], in0=jf[:], in1=eq[:],
                                            op=mybir.AluOpType.subtract)
                    ji = sm.tile([128, 1], mybir.dt.int32, tag="ji")
                    nc.vector.tensor_copy(ji[:], jf[:])
                    nc.sync.dma_start(idx_flat[bass.ts(iv, 128)], ji[:])

                if unroll <= 0:
                    for i in range(NBLK):
                        body(i)
                else:
                    with tc.For_i(0, NBLK, unroll, name=f"t{t}") as iv:
                        body(iv)
                        for k in range(1, unroll):
                            body(nc.snap(iv + k))
    nc.compile()
    return nc


def _diagm():
    m = np.zeros((128, 16), np.float32)
    for p in range(128):
        m[p, p % 16] = 1.0
    return m


def _prep(x, y):
    """x: queries [N,3], y: refs [M,3] -> xa [4,N], ya [4,M] (fp32)."""
    x = np.asarray(x, dtype=np.float32)
    y = np.asarray(y, dtype=np.float32)
    xa = np.concatenate([x, np.ones((x.shape[0], 1), np.float32)], axis=1).T
    yn = (y * y).sum(axis=1, dtype=np.float32)
    ya = np.concatenate([2.0 * y, -yn[:, None]], axis=1).T
    return np.ascontiguousarray(xa), np.ascontiguousarray(ya)


def kernel(xyz1, xyz2):
    from concourse.bass_utils import run_bass_kernel_spmd

    xyz1 = np.asarray(xyz1, dtype=np.float32)
    xyz2 = np.asarray(xyz2, dtype=np.float32)
    B = xyz1.shape[0]
    assert B == 8 and xyz1.shape[1] == NQ and xyz2.shape[1] == NR

    if "nc" not in _cache:
        _cache["nc"] = _build_nc()
    nc = _cache["nc"]

    in_maps = []
    for b in range(B):
        xa1, ya1 = _prep(xyz1[b], xyz2[b])
        xa2, ya2 = _prep(xyz2[b], xyz1[b])
        in_maps.append({"xa1": xa1, "ya1": ya1, "xa2": xa2, "ya2": ya2,
                        "diagm": _diagm()})

    res = run_bass_kernel_spmd(nc, in_maps, list(range(B)))
    idx1 = np.stack([np.asarray(res.results[b]["idx1"]) for b in range(B)])
    idx2 = np.stack([np.asarray(res.results[b]["idx2"]) for b in range(B)])
    return idx1.astype(np.int32), idx2.astype(np.int32)

